# revision 19
# baseline (speedup 1.0000x reference)
"""Trainium2 Bass kernel for nn_Attention_12000138625343.

Full multi-head attention layer (B=2, S=2048, E=1024, H=16, hd=64, interleaved
RoPE on q/k, non-causal softmax) run tensor-parallel over 8 NeuronCores:

  - heads sharded 2-per-core (w1 columns / qkv projection sharded),
  - x replicated, passed pre-transposed [E, B*S] so the contraction dim lands
    on SBUF partitions,
  - all matmul operands in bfloat16 (PSUM accumulation stays fp32): the PE
    runs out of the fp32 power-throttle regime, LDWEIGHTS and every DMA
    halve, and DVE element-wise ops hit their 2x/4x 16-bit modes,
  - scores computed transposed [k, q]; the two heads' K=64 score matmuls are
    packed into disjoint PE row-groups (concurrent), one exp instruction
    covers both heads' [128, 1024] PSUM block,
  - the Scalar (ACT) engine runs *only* the exp stream - every PSUM eviction
    lives on Pool (gpsimd) or DVE so softmax throughput is never stolen,
  - the v projection runs in its fast transposed orientation (N=512) and is
    flipped back to [k, hd] by PE transposes against an identity,
  - attn@v accumulates rolling per k-chunk with a ones-column appended to v
    producing the softmax denominator; the divide runs entirely off the
    TensorEngine (DVE reciprocal + DRAM-bounce broadcast DMA + DVE multiply),
  - batch-1 qkv projection / batch-0 output projection matmul chains are
    dribbled into the attention k-chunk loop so the in-order PE stream never
    starves the exp pipeline for long,
  - four bf16 AllToAlls (one per batch-half, each gated by that half's last
    softmax divide) convert the head sharding of the attention output o^T
    into row sharding; all but the final 0.25 MB collective hide under
    remaining compute, and each w2-projection block runs as soon as its
    half has arrived,
  - each core owns 2 x 128 rows of each batch; host reassembles.
"""

import math

import numpy as np

import concourse.bass as bass
import concourse.mybir as mybir
import concourse.tile as tile
from concourse import bacc
from concourse.bass_utils import run_bass_kernel_spmd
from concourse.masks import make_identity

B, S, E, H = 2, 2048, 1024, 16
HD = E // H  # 64
BASE = 10000.0
N_CORES = 8
HPC = H // N_CORES       # heads per core = 2
R = B * S                # 4096 flattened rows
RT = 512                 # rows per r-tile
NEC = E // 128           # 8 e-chunks of 128
QT = 512                 # q columns per q-tile
N_QT = S // QT           # 4 q-tiles per batch
KC = 128                 # k rows per k-chunk
N_KC = S // KC           # 16 k-chunks per batch
RPB = S // N_CORES       # rows per core per batch = 256

F32 = mybir.dt.float32
BF = mybir.dt.bfloat16
EXPF = mybir.ActivationFunctionType.Exp

_COMPILED = {}


def _build_nc():
    nc = bacc.Bacc("TRN2", target_bir_lowering=False, debug=False,
                   num_devices=N_CORES)

    xT = nc.dram_tensor("xT", [E, R], BF, kind="ExternalInput").ap()
    wqT = nc.dram_tensor("wqT", [E, 128], BF, kind="ExternalInput").ap()
    wkT = nc.dram_tensor("wkT", [E, 128], BF, kind="ExternalInput").ap()
    wvT = nc.dram_tensor("wvT", [E, 128], BF, kind="ExternalInput").ap()
    w2T = nc.dram_tensor("w2T", [E, E], BF, kind="ExternalInput").ap()
    cosT = nc.dram_tensor("cosT", [128, S], BF, kind="ExternalInput").ap()
    sinT = nc.dram_tensor("sinT", [128, S], BF, kind="ExternalInput").ap()
    p2T = nc.dram_tensor("p2T", [128, 128], BF, kind="ExternalInput").ap()
    out = nc.dram_tensor("out", [2 * RPB, E], F32, kind="ExternalOutput").ap()

    with tile.TileContext(nc) as tc:
        _emit(tc, nc, xT, wqT, wkT, wvT, w2T, cosT, sinT, p2T, out)
    nc.compile()
    return nc


def _emit(tc, nc, xT, wqT, wkT, wvT, w2T, cosT, sinT, p2T, out):
    import contextlib
    ctx = contextlib.ExitStack()
    consts = ctx.enter_context(tc.tile_pool(name="consts", bufs=1))
    xtp = ctx.enter_context(tc.tile_pool(name="xtp", bufs=2))
    qkp = ctx.enter_context(tc.tile_pool(name="qkp", bufs=1))
    rawp = ctx.enter_context(tc.tile_pool(name="rawp", bufs=2))
    tmpp = ctx.enter_context(tc.tile_pool(name="tmpp", bufs=2))
    vp = ctx.enter_context(tc.tile_pool(name="vp", bufs=1))
    pp = ctx.enter_context(tc.tile_pool(name="pp", bufs=7))
    smallp = ctx.enter_context(tc.tile_pool(name="smallp", bufs=2))
    dramp = ctx.enter_context(tc.tile_pool(name="dramp", bufs=1, space="DRAM"))
    # PSUM budget (8 banks): qkv-shared 2 + sps 2 x 2 + av 2 = 8
    ps_qkv = ctx.enter_context(tc.tile_pool(name="ps_qkv", bufs=2, space="PSUM"))
    ps_sps = ctx.enter_context(tc.tile_pool(name="ps_sps", bufs=2, space="PSUM"))
    ps_av = ctx.enter_context(tc.tile_pool(name="ps_av", bufs=2, space="PSUM"))

    # ---- batched constant loads (single DMA each, all bf16); q weights +
    # the first half of the first x tile go first so the first matmul can
    # start a few us in ----
    wq_all = consts.tile([128, NEC, 128], BF, tag="wq", name="wq_all")
    nc.sync.dma_start(out=wq_all[:], in_=wqT.rearrange("(c p) f -> p c f", p=128))
    xt0 = xtp.tile([128, NEC, RT], BF, tag="xt", bufs=3, name="xt_0")
    xr = xT.rearrange("(c p) r -> p c r", p=128)
    nc.gpsimd.dma_start(out=xt0[:, 0:4, :], in_=xr[:, 0:4, 0:RT])
    nc.gpsimd.dma_start(out=xt0[:, 4:NEC, :], in_=xr[:, 4:NEC, 0:RT])
    wk_all = consts.tile([128, NEC, 128], BF, tag="wk", name="wk_all")
    nc.gpsimd.dma_start(out=wk_all[:], in_=wkT.rearrange("(c p) f -> p c f", p=128))
    wv_all = consts.tile([128, NEC, 128], BF, tag="wv", name="wv_all")
    nc.gpsimd.dma_start(out=wv_all[:], in_=wvT.rearrange("(c p) f -> p c f", p=128))
    p2_sb = consts.tile([128, 128], BF, tag="p2", name="p2_sb")
    nc.gpsimd.dma_start(out=p2_sb[:], in_=p2T[:, :])

    cos_sb = consts.tile([128, S], BF, tag="cos", name="cos_sb")
    nc.sync.dma_start(out=cos_sb[:], in_=cosT[:, :])
    sin_sb = consts.tile([128, S], BF, tag="sin", name="sin_sb")
    nc.sync.dma_start(out=sin_sb[:], in_=sinT[:, :])
    ones_f32 = consts.tile([1, 64], F32, tag="ones32", name="ones_f32")
    nc.vector.memset(ones_f32[:], 1.0)
    id_sb = consts.tile([128, 128], BF, tag="idm", name="id_sb")
    make_identity(nc, id_sb[:])

    # A2A buffers, one pair per (batch, half): [8 chunks, 128 e-rows, 128 rows]
    # half 0 carries s in [128j, 128j+128) (ready after q-tile 1),
    # half 1 carries s in [1024+128j, ...) (ready after q-tile 3).
    send_d = {(b, hf): dramp.tile([N_CORES, 128, 128], BF, name=f"send{b}{hf}")
              for b in range(B) for hf in range(2)}
    recv_d = {(b, hf): dramp.tile([N_CORES, 128, 128], BF, name=f"recv{b}{hf}")
              for b in range(B) for hf in range(2)}

    qT_sb, kT_sb, v_sb = {}, {}, {}
    w2_sb = {}

    def emit_xt_load(rt):
        if rt == 0:
            return xt0
        t = xtp.tile([128, NEC, RT], BF, tag="xt", bufs=3, name=f"xt_{rt}")
        nc.gpsimd.dma_start(
            out=t[:],
            in_=xr[:, :, rt * RT:(rt + 1) * RT])
        return t

    def qkv_chains(rt, get_xt):
        """Return a list of closures, each emitting one matmul chain (+ its
        epilogue) for r-tile rt. Callers dribble these between attention
        steps to keep the in-order PE stream dense but never monolithic.
        get_xt is called at chain-emission time so the x load DMA can be
        prefetched by an earlier dribble entry."""
        b, st = rt // N_QT, (rt % N_QT) * RT

        if b not in qT_sb:
            qT_sb[b] = qkp.tile([128, S], BF, tag=f"qT{b}", name=f"qT{b}")
            kT_sb[b] = qkp.tile([128, S], BF, tag=f"kT{b}", name=f"kT{b}")

        def qk_chain(kind, w_all, dst):
            state = {}
            def emit_a():
                xt = get_xt()
                acc = ps_qkv.tile([128, RT], F32, tag="qkv",
                                  name=f"{kind}acc{rt}")
                for ec in range(4):
                    nc.tensor.matmul(acc[:], w_all[:, ec, :], xt[:, ec, :],
                                     start=(ec == 0), stop=False)
                state["acc"] = acc
            def emit_b():
                xt = get_xt()
                acc = state.pop("acc")
                for ec in range(4, NEC):
                    nc.tensor.matmul(acc[:], w_all[:, ec, :], xt[:, ec, :],
                                     start=False, stop=(ec == NEC - 1))
                raw = rawp.tile([128, RT], BF, tag="raw",
                                name=f"{kind}raw{rt}")
                nc.vector.tensor_copy(raw[:], acc[:])
                rot = ps_qkv.tile([128, RT], F32, tag="qkv",
                                  name=f"{kind}rot{rt}")
                nc.tensor.matmul(rot[:], p2_sb[:], raw[:], start=True, stop=True)
                t1 = tmpp.tile([128, RT], BF, tag="ropet", name=f"{kind}t1_{rt}")
                nc.vector.tensor_mul(t1[:], raw[:], cos_sb[:, st:st + RT])
                t2 = tmpp.tile([128, RT], BF, tag="ropet", name=f"{kind}t2_{rt}")
                nc.vector.tensor_mul(t2[:], rot[:], sin_sb[:, st:st + RT])
                nc.vector.tensor_add(dst[:, st:st + RT], t1[:], t2[:])
            return [emit_a, emit_b]

        vstate = {}

        def v_head_chain(half):
            # v^T = wv.T @ x computed at full rate (N=512), half the e-chunks
            # per pop; the PE transpose in v_tail_chain flips it back to the
            # [k, hd] layout attn@v needs.
            def emit():
                xt = get_xt()
                if half == 0:
                    vacc = ps_qkv.tile([128, RT], F32, tag="qkv",
                                       name=f"vTacc{rt}")
                    vstate["ps"] = vacc
                vacc = vstate["ps"]
                for ec in range(4 * half, 4 * half + 4):
                    nc.tensor.matmul(vacc[:], wv_all[:, ec, :], xt[:, ec, :],
                                     start=(ec == 0), stop=(ec == NEC - 1))
                if half == 1:
                    vts = rawp.tile([128, RT], BF, tag="raw",
                                    name=f"vts{rt}")
                    nc.vector.tensor_copy(vts[:], vstate.pop("ps")[:])
                    vstate["sb"] = vts
            return emit

        def v_tail_chain(pair):
            def emit():
                vts = vstate["sb"]
                for sub in (2 * pair, 2 * pair + 1):
                    vtr = ps_qkv.tile([128, 128], BF, tag="qkv",
                                      name=f"vtr{rt}_{sub}")
                    nc.tensor.transpose(
                        vtr[:], vts[:, sub * 128:(sub + 1) * 128], id_sb[:])
                    kc = (rt % N_QT) * 4 + sub
                    # both heads' [k, 64] blocks + their ones columns live in
                    # one [128, 130] tile: a single strided-AP copy and a
                    # single strided memset replace 4 per-head DVE ops
                    vt = vp.tile([128, 130], BF, tag=f"v{b}{kc}",
                                 name=f"v{b}{kc}")
                    vt_h = vt[:, 0:130].rearrange("p (h c) -> p h c", c=65)
                    nc.vector.tensor_copy(
                        vt_h[:, :, 0:64],
                        vtr[:, 0:128].rearrange("p (h c) -> p h c", c=64))
                    nc.vector.memset(vt_h[:, :, 64:65], 1.0)
                    for h in range(HPC):
                        v_sb[(b, h, kc)] = vt[:, 65 * h:65 * h + 65]
            return emit

        return qk_chain("q", wq_all, qT_sb[b]) + \
               qk_chain("k", wk_all, kT_sb[b]) + \
               [v_head_chain(0), v_head_chain(1),
                v_tail_chain(0), v_tail_chain(1)]

    def proj_chains(b, hf):
        """Output projection for my 128 rows of (batch b, half hf).
        The recv load is emitted lazily by the first chain so that building
        the chain list never precedes the collective's emission."""
        state0 = {}
        def get_recv():
            if "t" not in state0:
                t = xtp.tile([128, NEC, 128], BF, tag="recv", bufs=2,
                             name=f"recv{b}{hf}")
                nc.gpsimd.dma_start(
                    out=t[:], in_=recv_d[(b, hf)].rearrange("c p r -> p c r"))
                state0["t"] = t
            return state0["t"]
        chains = []
        for rblk in [hf]:
            for ft in range(2):
                state = {}
                def emit_a(rblk=rblk, ft=ft, state=state):
                    recv_sb = get_recv()
                    # qkv psum tag: free during attention (projection is done)
                    ops = ps_qkv.tile([128, 512], F32, tag="qkv",
                                      name=f"ops{b}_{rblk}_{ft}")
                    for ec in range(4):
                        nc.tensor.matmul(
                            ops[:],
                            recv_sb[:, ec, :],
                            w2_sb[0][:, ec, ft * 512:(ft + 1) * 512],
                            start=(ec == 0), stop=False)
                    state["ops"] = ops
                def emit_b(rblk=rblk, ft=ft, state=state):
                    recv_sb = get_recv()
                    ops = state.pop("ops")
                    for ec in range(4, NEC):
                        nc.tensor.matmul(
                            ops[:],
                            recv_sb[:, ec, :],
                            w2_sb[0][:, ec, ft * 512:(ft + 1) * 512],
                            start=False, stop=(ec == NEC - 1))
                    ot = tmpp.tile([128, 512], F32, tag="ropet",
                                   name=f"ot{b}_{rblk}_{ft}")
                    nc.vector.tensor_copy(ot[:], ops[:])
                    # out rows: [b0h0, b0h1, b1h0, b1h1] blocks of 128
                    ob = 2 * b + rblk
                    nc.sync.dma_start(
                        out=out[ob * 128:(ob + 1) * 128,
                                ft * 512:(ft + 1) * 512],
                        in_=ot[:])
                chains.append(emit_a)
                chains.append(emit_b)
        return chains

    def emit_divide(b, qt, avs):
        """Divide by the softmax denominator (row 64 of av) and stage into
        the A2A send buffer. PE-free, and the reciprocal is computed on a
        [128, 4] partition-spread layout (a [1, 512] DVE reciprocal costs
        ~3.3us on one partition; spread it costs ~0.1us). Both heads' PSUM
        evictions run first so the next q-tile's attn@v never stalls on
        the divide chain. All DMAs on the sync HWDGE queue so the
        collective sitting on the gpsimd queue can never block them."""
        oraws = []
        for h in range(HPC):
            # evict the accumulator to SBUF immediately: releases the PSUM
            # slot so the next q-tile's attn@v never waits on this divide
            oraw = smallp.tile([65, QT], BF, tag="oraw", name=f"oraw{b}{h}{qt}")
            nc.vector.tensor_copy(oraw[:], avs[h][:])
            oraws.append(oraw)
        for h in range(HPC):
            oraw = oraws[h]
            den_d = dramp.tile([1, QT], BF, tag="dend", bufs=4,
                               name=f"dend{b}{h}{qt}")
            nc.sync.dma_start(out=den_d[:], in_=oraw[64:65, :])
            den_sp = den_d.rearrange("o (p c) -> (o p) c", p=128)
            den128 = smallp.tile([128, 4], BF, tag="den128",
                                 name=f"den128{b}{h}{qt}")
            nc.sync.dma_start(out=den128[:], in_=den_sp)
            rcp128 = smallp.tile([128, 4], BF, tag="rcp128",
                                 name=f"rcp128{b}{h}{qt}")
            with nc.allow_low_precision(reason="bf16 softmax denominator"):
                nc.vector.reciprocal(rcp128[:], den128[:])
            rcp_d = dramp.tile([1, QT], BF, tag="rcpd", bufs=4,
                               name=f"rcpd{b}{h}{qt}")
            nc.sync.dma_start(out=rcp_d.rearrange("o (p c) -> (o p) c", p=128),
                              in_=rcp128[:])
            bcs = smallp.tile([64, QT], BF, tag="bcs", name=f"bcs{b}{h}{qt}")
            bcast = bass.AP(tensor=rcp_d.tensor, offset=rcp_d.offset,
                            ap=[[0, 64]] + list(rcp_d.ap[1:]))
            nc.sync.dma_start(out=bcs[:], in_=bcast)
            odiv = smallp.tile([64, QT], BF, tag="odiv", name=f"odiv{b}{h}{qt}")
            nc.vector.tensor_mul(odiv[:], oraw[0:64, :], bcs[:])
            # q-tile qt covers s in [512qt, 512qt+512): half hf = qt // 2,
            # destination cores j = 4*(qt%2) .. +4, 128 columns each
            hf = qt // 2
            for jj in range(4):
                j = 4 * (qt % 2) + jj
                nc.sync.dma_start(
                    out=send_d[(b, hf)][j, h * 64:(h + 1) * 64, :],
                    in_=odiv[:, jj * 128:(jj + 1) * 128])

    def emit_attention_batch(b, dribble):
        """All 4 q-tiles of a batch as one rolling pipeline over 64+LAG
        (qt, kc) units: scores+exp lead, attn@v trails by LAG units, the
        divide chain fires as each q-tile's accumulation completes.  One
        dribble chain (qkv projection / output projection) is popped every
        other unit to keep the in-order PE stream dense."""
        scale = 1.0 / math.sqrt(HD)
        NU = N_QT * N_KC
        LAG = 5
        pts = {}
        avs = {}
        for u in range(NU + LAG):
            if u < NU:
                qt, kc = divmod(u, N_KC)
                if kc == 0:
                    avs[qt] = [ps_av.tile([65, QT], F32, tag="av",
                                          name=f"av{b}{h}{qt}")
                               for h in range(HPC)]
                sps = ps_sps.tile([128, 2 * QT], F32, tag="sps",
                                  name=f"s{b}{qt}_{kc}")
                for h in range(HPC):
                    hof = h * 64
                    nc.tensor.matmul(
                        sps[:, h * QT:(h + 1) * QT],
                        kT_sb[b][hof:hof + 64, kc * KC:(kc + 1) * KC],
                        qT_sb[b][hof:hof + 64, qt * QT:(qt + 1) * QT],
                        start=True, stop=True)
                pt = pp.tile([128, 2 * QT], BF, tag="p", name=f"p{b}{qt}_{kc}")
                nc.scalar.activation(pt[:], sps[:], EXPF, scale=scale)
                pts[u] = pt
            if u >= LAG:
                j = u - LAG
                qt2, kc2 = divmod(j, N_KC)
                for h in range(HPC):
                    nc.tensor.matmul(avs[qt2][h][:], v_sb[(b, h, kc2)],
                                     pts[j][:, h * QT:(h + 1) * QT],
                                     start=(kc2 == 0), stop=(kc2 == N_KC - 1))
                del pts[j]
                if kc2 == N_KC - 1:
                    emit_divide(b, qt2, avs.pop(qt2))
                    if qt2 == 1:
                        emit_a2a(b, 0)
            # pop every dribble entry whose target unit has arrived
            while dribble and dribble[0][0] <= u:
                dribble.pop(0)[1]()

    def emit_a2a(b, hf):
        nc.gpsimd.collective_compute(
            "AllToAll", mybir.AluOpType.bypass,
            replica_groups=[list(range(N_CORES))],
            ins=[send_d[(b, hf)].opt()], outs=[recv_d[(b, hf)].opt()])

    # ---------------- emission ----------------
    # r-tile 0 of batch 0 runs alone; batch-0 attention starts right after
    # (its first 4 units only need r-tile 0's q/k/v), with r-tiles 1-3 and
    # then batch 1's qkv dribbled into the unit loop at 2 chains/unit so
    # each r-tile completes just before the units that consume it.
    holders = {0: xt0}

    def prefetch(rt):
        def fn():
            if rt not in holders:
                holders[rt] = emit_xt_load(rt)
        return fn

    def get_xt(rt):
        def g():
            if rt not in holders:
                holders[rt] = emit_xt_load(rt)
            return holders[rt]
        return g

    for chain in qkv_chains(0, get_xt(0)):
        chain()
    # warm the collective path (cold-start ~8us); emitted here so the wait on
    # the gpsimd queue never delays the critical first x/weight loads
    cwu_s = dramp.tile([N_CORES, 8], F32, tag="cwus", name="cwu_s")
    cwu_r = dramp.tile([N_CORES, 8], F32, tag="cwur", name="cwu_r")
    nc.sync.dma_start(out=cwu_s.rearrange("c r -> (c r)")[None, :],
                      in_=ones_f32[0:1, 0:64])
    nc.gpsimd.collective_compute(
        "AllToAll", mybir.AluOpType.bypass,
        replica_groups=[list(range(N_CORES))],
        ins=[cwu_s.opt()], outs=[cwu_r.opt()])

    # r-tile 1's x load starts before the w2 load grabs the queue: its first
    # consumers are the dribbled chains at units 0-1
    prefetch(1)()
    # w2 load early: 2 MB bf16, overlaps the batch-0 attention stretch
    w2_sb[0] = consts.tile([128, NEC, E], BF, tag="w2", name="w2_all")
    nc.gpsimd.dma_start(out=w2_sb[0][:],
                        in_=w2T.rearrange("(c p) f -> p c f", p=128))
    # batch-0 attention with b0 r-tiles 1-3 then batch-1 qkv dribbled in
    dribble = []
    for rt in (2, 3):
        dribble.append((4 * (rt - 1) - 3, prefetch(rt)))
    for rt in (1, 2, 3):
        for i, c in enumerate(qkv_chains(rt, get_xt(rt))):
            dribble.append((4 * (rt - 1) + i // 2, c))
    for k, rt in enumerate((4, 5, 6, 7)):
        base = 12 + 14 * k
        dribble.append((base - 3, prefetch(rt)))
        for i, c in enumerate(qkv_chains(rt, get_xt(rt))):
            dribble.append((base + i * 14 // 8, c))
    dribble.sort(key=lambda e: e[0])
    emit_attention_batch(0, dribble)
    for _, chain in dribble:
        chain()
    del dribble[:]
    emit_a2a(0, 1)                     # second half, fires at batch-0 end

    # both batch-0 halves complete early in batch-1 attention;
    # batch-1 half 0's A2A fires mid-batch, its projection runs at the tail
    dribble = [(24 + 2 * i, c)
               for i, c in enumerate(proj_chains(0, 0) + proj_chains(0, 1))]
    dribble += [(58 + 2 * i, c) for i, c in enumerate(proj_chains(1, 0))]
    emit_attention_batch(1, dribble)
    for _, chain in dribble:
        chain()
    emit_a2a(1, 1)
    for chain in proj_chains(1, 1):
        chain()
    ctx.close()


def _host_prep(x, w1, w2):
    import ml_dtypes
    bf16 = ml_dtypes.bfloat16
    x = np.asarray(x, dtype=np.float32)
    w1 = np.asarray(w1, dtype=np.float32)
    w2 = np.asarray(w2, dtype=np.float32)

    xT = np.ascontiguousarray(x.reshape(R, E).T.astype(bf16))   # [E, R]
    w2T = np.ascontiguousarray(w2.T.astype(bf16))               # [E, E]

    theta = 1.0 / (BASE ** (np.arange(0, HD, 2, dtype=np.float32) / HD))
    enc = np.arange(S, dtype=np.float32)[:, None] * theta[None, :]
    enc = np.repeat(enc, 2, axis=-1)                      # [s, 64]
    cos1 = np.cos(enc).T.astype(np.float32)               # [64, S]
    sin1 = np.sin(enc).T.astype(np.float32)
    cosT = np.ascontiguousarray(np.concatenate([cos1, cos1], axis=0).astype(bf16))
    sinT = np.ascontiguousarray(np.concatenate([sin1, sin1], axis=0).astype(bf16))

    m64 = np.zeros((HD, HD), dtype=np.float32)
    for i in range(HD // 2):
        m64[2 * i, 2 * i + 1] = -1.0
        m64[2 * i + 1, 2 * i] = 1.0
    m128 = np.zeros((128, 128), dtype=np.float32)
    m128[:64, :64] = m64
    m128[64:, 64:] = m64
    p2T = np.ascontiguousarray(m128.T.astype(bf16))

    in_maps = []
    for c in range(N_CORES):
        hA, hB = HPC * c, HPC * c + 1
        def rows(base):
            return np.concatenate(
                [w1[base + hA * HD: base + (hA + 1) * HD, :],
                 w1[base + hB * HD: base + (hB + 1) * HD, :]], axis=0)
        in_maps.append({
            "xT": xT,
            "wqT": np.ascontiguousarray(rows(0).T.astype(bf16)),
            "wkT": np.ascontiguousarray(rows(E).T.astype(bf16)),
            "wvT": np.ascontiguousarray(rows(2 * E).T.astype(bf16)),
            "w2T": w2T,
            "cosT": cosT,
            "sinT": sinT,
            "p2T": p2T,
        })
    return in_maps


def kernel(x, w1, w2, _trace=False):
    if "nc" not in _COMPILED:
        _COMPILED["nc"] = _build_nc()
    nc = _COMPILED["nc"]
    in_maps = _host_prep(x, w1, w2)
    res = run_bass_kernel_spmd(nc, in_maps, core_ids=list(range(N_CORES)),
                               trace=_trace)
    _COMPILED["last_result"] = res
    # core c returns [512, E] as four 128-row blocks:
    # [b0 s=128c.., b0 s=1024+128c.., b1 s=128c.., b1 s=1024+128c..]
    full = np.empty((B, S, E), dtype=np.float32)
    for c in range(N_CORES):
        blk = res.results[c]["out"]
        full[0, 128 * c:128 * (c + 1)] = blk[0:128]
        full[0, 1024 + 128 * c:1024 + 128 * (c + 1)] = blk[128:256]
        full[1, 128 * c:128 * (c + 1)] = blk[256:384]
        full[1, 1024 + 128 * c:1024 + 128 * (c + 1)] = blk[384:512]
    return full


# revision 21
# speedup vs baseline: 1.0457x; 1.0457x over previous
"""Trainium2 Bass kernel for nn_Attention_12000138625343.

Full multi-head attention layer (B=2, S=2048, E=1024, H=16, hd=64, interleaved
RoPE on q/k, non-causal softmax) run tensor-parallel over 8 NeuronCores:

  - heads sharded 2-per-core (w1 columns / qkv projection sharded),
  - x replicated, passed pre-transposed [E, B*S] so the contraction dim lands
    on SBUF partitions,
  - all matmul operands in bfloat16 (PSUM accumulation stays fp32): the PE
    runs out of the fp32 power-throttle regime, LDWEIGHTS and every DMA
    halve, and DVE element-wise ops hit their 2x/4x 16-bit modes,
  - scores computed transposed [k, q]; the two heads' K=64 score matmuls are
    packed into disjoint PE row-groups (concurrent), one exp instruction
    covers both heads' [128, 1024] PSUM block,
  - the Scalar (ACT) engine runs *only* the exp stream - every PSUM eviction
    lives on Pool (gpsimd) or DVE so softmax throughput is never stolen,
  - the v projection runs in its fast transposed orientation (N=512) and is
    flipped back to [k, hd] by PE transposes against an identity,
  - attn@v accumulates rolling per k-chunk with a ones-column appended to v
    producing the softmax denominator; the divide runs entirely off the
    TensorEngine (DVE reciprocal + DRAM-bounce broadcast DMA + DVE multiply),
  - batch-1 qkv projection / batch-0 output projection matmul chains are
    dribbled into the attention k-chunk loop so the in-order PE stream never
    starves the exp pipeline for long,
  - four bf16 AllToAlls (one per batch-half, each gated by that half's last
    softmax divide) convert the head sharding of the attention output o^T
    into row sharding; all but the final 0.25 MB collective hide under
    remaining compute, and each w2-projection block runs as soon as its
    half has arrived,
  - each core owns 2 x 128 rows of each batch; host reassembles.
"""

import math

import numpy as np

import concourse.bass as bass
import concourse.mybir as mybir
import concourse.tile as tile
from concourse import bacc
from concourse.bass_utils import run_bass_kernel_spmd
from concourse.masks import make_identity

B, S, E, H = 2, 2048, 1024, 16
HD = E // H  # 64
BASE = 10000.0
N_CORES = 8
HPC = H // N_CORES       # heads per core = 2
R = B * S                # 4096 flattened rows
RT = 512                 # rows per r-tile
NEC = E // 128           # 8 e-chunks of 128
QT = 512                 # q columns per q-tile
N_QT = S // QT           # 4 q-tiles per batch
KC = 128                 # k rows per k-chunk
N_KC = S // KC           # 16 k-chunks per batch
RPB = S // N_CORES       # rows per core per batch = 256

F32 = mybir.dt.float32
BF = mybir.dt.bfloat16
EXPF = mybir.ActivationFunctionType.Exp

_COMPILED = {}


def _build_nc():
    nc = bacc.Bacc("TRN2", target_bir_lowering=False, debug=False,
                   num_devices=N_CORES)

    xT = nc.dram_tensor("xT", [E, R], BF, kind="ExternalInput").ap()
    wqT = nc.dram_tensor("wqT", [E, 128], BF, kind="ExternalInput").ap()
    wkT = nc.dram_tensor("wkT", [E, 128], BF, kind="ExternalInput").ap()
    wvT = nc.dram_tensor("wvT", [E, 128], BF, kind="ExternalInput").ap()
    w2T = nc.dram_tensor("w2T", [E, E], BF, kind="ExternalInput").ap()
    cosT = nc.dram_tensor("cosT", [128, S], BF, kind="ExternalInput").ap()
    sinT = nc.dram_tensor("sinT", [128, S], BF, kind="ExternalInput").ap()
    p2T = nc.dram_tensor("p2T", [128, 128], BF, kind="ExternalInput").ap()
    out = nc.dram_tensor("out", [2 * RPB, E], F32, kind="ExternalOutput").ap()

    with tile.TileContext(nc) as tc:
        _emit(tc, nc, xT, wqT, wkT, wvT, w2T, cosT, sinT, p2T, out)
    nc.compile()
    return nc


def _emit(tc, nc, xT, wqT, wkT, wvT, w2T, cosT, sinT, p2T, out):
    import contextlib
    ctx = contextlib.ExitStack()
    consts = ctx.enter_context(tc.tile_pool(name="consts", bufs=1))
    xtp = ctx.enter_context(tc.tile_pool(name="xtp", bufs=2))
    qkp = ctx.enter_context(tc.tile_pool(name="qkp", bufs=1))
    rawp = ctx.enter_context(tc.tile_pool(name="rawp", bufs=2))
    tmpp = ctx.enter_context(tc.tile_pool(name="tmpp", bufs=2))
    vp = ctx.enter_context(tc.tile_pool(name="vp", bufs=1))
    pp = ctx.enter_context(tc.tile_pool(name="pp", bufs=7))
    smallp = ctx.enter_context(tc.tile_pool(name="smallp", bufs=2))
    dramp = ctx.enter_context(tc.tile_pool(name="dramp", bufs=1, space="DRAM"))
    # PSUM budget (8 banks): qkv-shared 2 + sps 2 x 2 + av 2 = 8
    ps_qkv = ctx.enter_context(tc.tile_pool(name="ps_qkv", bufs=2, space="PSUM"))
    ps_sps = ctx.enter_context(tc.tile_pool(name="ps_sps", bufs=2, space="PSUM"))
    ps_av = ctx.enter_context(tc.tile_pool(name="ps_av", bufs=2, space="PSUM"))

    # ---- batched constant loads (single DMA each, all bf16); q weights +
    # the first half of the first x tile go first so the first matmul can
    # start a few us in ----
    wq_all = consts.tile([128, NEC, 128], BF, tag="wq", name="wq_all")
    nc.sync.dma_start(out=wq_all[:], in_=wqT.rearrange("(c p) f -> p c f", p=128))
    xt0 = xtp.tile([128, NEC, RT], BF, tag="xt", bufs=3, name="xt_0")
    xr = xT.rearrange("(c p) r -> p c r", p=128)
    nc.gpsimd.dma_start(out=xt0[:, 0:4, :], in_=xr[:, 0:4, 0:RT])
    nc.gpsimd.dma_start(out=xt0[:, 4:NEC, :], in_=xr[:, 4:NEC, 0:RT])
    wk_all = consts.tile([128, NEC, 128], BF, tag="wk", name="wk_all")
    nc.gpsimd.dma_start(out=wk_all[:], in_=wkT.rearrange("(c p) f -> p c f", p=128))
    wv_all = consts.tile([128, NEC, 128], BF, tag="wv", name="wv_all")
    nc.gpsimd.dma_start(out=wv_all[:], in_=wvT.rearrange("(c p) f -> p c f", p=128))
    p2_sb = consts.tile([128, 128], BF, tag="p2", name="p2_sb")
    nc.gpsimd.dma_start(out=p2_sb[:], in_=p2T[:, :])

    cos_sb = consts.tile([128, S], BF, tag="cos", name="cos_sb")
    nc.sync.dma_start(out=cos_sb[:], in_=cosT[:, :])
    sin_sb = consts.tile([128, S], BF, tag="sin", name="sin_sb")
    nc.sync.dma_start(out=sin_sb[:], in_=sinT[:, :])
    ones_f32 = consts.tile([1, 64], F32, tag="ones32", name="ones_f32")
    nc.vector.memset(ones_f32[:], 1.0)
    id_sb = consts.tile([128, 128], BF, tag="idm", name="id_sb")
    make_identity(nc, id_sb[:])

    # A2A buffers, one pair per (batch, half): [8 chunks, 128 e-rows, 128 rows]
    # half 0 carries s in [128j, 128j+128) (ready after q-tile 1),
    # half 1 carries s in [1024+128j, ...) (ready after q-tile 3).
    send_d = {(b, hf): dramp.tile([N_CORES, 128, 128], BF, name=f"send{b}{hf}")
              for b in range(B) for hf in range(2)}
    recv_d = {(b, hf): dramp.tile([N_CORES, 128, 128], BF, name=f"recv{b}{hf}")
              for b in range(B) for hf in range(2)}

    qT_sb, kT_sb, v_sb = {}, {}, {}
    w2_sb = {}

    def emit_xt_load(rt):
        if rt == 0:
            return xt0
        t = xtp.tile([128, NEC, RT], BF, tag="xt", bufs=3, name=f"xt_{rt}")
        nc.gpsimd.dma_start(
            out=t[:],
            in_=xr[:, :, rt * RT:(rt + 1) * RT])
        return t

    def qkv_chains(rt, get_xt):
        """Return a list of closures, each emitting one matmul chain (+ its
        epilogue) for r-tile rt. Callers dribble these between attention
        steps to keep the in-order PE stream dense but never monolithic.
        get_xt is called at chain-emission time so the x load DMA can be
        prefetched by an earlier dribble entry."""
        b, st = rt // N_QT, (rt % N_QT) * RT

        if b not in qT_sb:
            qT_sb[b] = qkp.tile([128, S], BF, tag=f"qT{b}", name=f"qT{b}")
            kT_sb[b] = qkp.tile([128, S], BF, tag=f"kT{b}", name=f"kT{b}")

        def qk_chain(kind, w_all, dst):
            state = {}
            def emit_a():
                xt = get_xt()
                acc = ps_qkv.tile([128, RT], F32, tag="qkv",
                                  name=f"{kind}acc{rt}")
                for ec in range(4):
                    nc.tensor.matmul(acc[:], w_all[:, ec, :], xt[:, ec, :],
                                     start=(ec == 0), stop=False)
                state["acc"] = acc
            def emit_b():
                xt = get_xt()
                acc = state.pop("acc")
                for ec in range(4, NEC):
                    nc.tensor.matmul(acc[:], w_all[:, ec, :], xt[:, ec, :],
                                     start=False, stop=(ec == NEC - 1))
                raw = rawp.tile([128, RT], BF, tag="raw",
                                name=f"{kind}raw{rt}")
                nc.vector.tensor_copy(raw[:], acc[:])
                rot = ps_qkv.tile([128, RT], F32, tag="qkv",
                                  name=f"{kind}rot{rt}")
                nc.tensor.matmul(rot[:], p2_sb[:], raw[:], start=True, stop=True)
                t1 = tmpp.tile([128, RT], BF, tag="ropet", name=f"{kind}t1_{rt}")
                nc.vector.tensor_mul(t1[:], raw[:], cos_sb[:, st:st + RT])
                t2 = tmpp.tile([128, RT], BF, tag="ropet", name=f"{kind}t2_{rt}")
                nc.vector.tensor_mul(t2[:], rot[:], sin_sb[:, st:st + RT])
                nc.vector.tensor_add(dst[:, st:st + RT], t1[:], t2[:])
            return [emit_a, emit_b]

        vstate = {}

        def v_head_chain(half):
            # v^T = wv.T @ x computed at full rate (N=512), half the e-chunks
            # per pop; the PE transpose in v_tail_chain flips it back to the
            # [k, hd] layout attn@v needs.
            def emit():
                xt = get_xt()
                if half == 0:
                    vacc = ps_qkv.tile([128, RT], F32, tag="qkv",
                                       name=f"vTacc{rt}")
                    vstate["ps"] = vacc
                vacc = vstate["ps"]
                for ec in range(4 * half, 4 * half + 4):
                    nc.tensor.matmul(vacc[:], wv_all[:, ec, :], xt[:, ec, :],
                                     start=(ec == 0), stop=(ec == NEC - 1))
                if half == 1:
                    vts = rawp.tile([128, RT], BF, tag="raw",
                                    name=f"vts{rt}")
                    nc.vector.tensor_copy(vts[:], vstate.pop("ps")[:])
                    vstate["sb"] = vts
            return emit

        def v_tail_chain(pair):
            def emit():
                vts = vstate["sb"]
                for sub in (2 * pair, 2 * pair + 1):
                    vtr = ps_qkv.tile([128, 128], BF, tag="qkv",
                                      name=f"vtr{rt}_{sub}")
                    nc.tensor.transpose(
                        vtr[:], vts[:, sub * 128:(sub + 1) * 128], id_sb[:])
                    kc = (rt % N_QT) * 4 + sub
                    # both heads' [k, 64] blocks + their ones columns live in
                    # one [128, 130] tile: a single strided-AP copy and a
                    # single strided memset replace 4 per-head DVE ops
                    vt = vp.tile([128, 130], BF, tag=f"v{b}{kc}",
                                 name=f"v{b}{kc}")
                    vt_h = vt[:, 0:130].rearrange("p (h c) -> p h c", c=65)
                    nc.vector.tensor_copy(
                        vt_h[:, :, 0:64],
                        vtr[:, 0:128].rearrange("p (h c) -> p h c", c=64))
                    nc.vector.memset(vt_h[:, :, 64:65], 1.0)
                    for h in range(HPC):
                        v_sb[(b, h, kc)] = vt[:, 65 * h:65 * h + 65]
            return emit

        return qk_chain("q", wq_all, qT_sb[b]) + \
               qk_chain("k", wk_all, kT_sb[b]) + \
               [v_head_chain(0), v_head_chain(1),
                v_tail_chain(0), v_tail_chain(1)]

    def proj_chains(b, hf):
        """Output projection for my 128 rows of (batch b, half hf).
        The recv load is emitted lazily by the first chain so that building
        the chain list never precedes the collective's emission."""
        state0 = {}
        def get_recv():
            if "t" not in state0:
                t = xtp.tile([128, NEC, 128], BF, tag="recv", bufs=2,
                             name=f"recv{b}{hf}")
                nc.gpsimd.dma_start(
                    out=t[:], in_=recv_d[(b, hf)].rearrange("c p r -> p c r"))
                state0["t"] = t
            return state0["t"]
        chains = []
        for rblk in [hf]:
            for ft in range(2):
                state = {}
                def emit_a(rblk=rblk, ft=ft, state=state):
                    recv_sb = get_recv()
                    # qkv psum tag: free during attention (projection is done)
                    ops = ps_qkv.tile([128, 512], F32, tag="qkv",
                                      name=f"ops{b}_{rblk}_{ft}")
                    for ec in range(4):
                        nc.tensor.matmul(
                            ops[:],
                            recv_sb[:, ec, :],
                            w2_sb[0][:, ec, ft * 512:(ft + 1) * 512],
                            start=(ec == 0), stop=False)
                    state["ops"] = ops
                def emit_b(rblk=rblk, ft=ft, state=state):
                    recv_sb = get_recv()
                    ops = state.pop("ops")
                    for ec in range(4, NEC):
                        nc.tensor.matmul(
                            ops[:],
                            recv_sb[:, ec, :],
                            w2_sb[0][:, ec, ft * 512:(ft + 1) * 512],
                            start=False, stop=(ec == NEC - 1))
                    ot = tmpp.tile([128, 512], F32, tag="ropet",
                                   name=f"ot{b}_{rblk}_{ft}")
                    nc.vector.tensor_copy(ot[:], ops[:])
                    # out rows: [b0h0, b0h1, b1h0, b1h1] blocks of 128
                    ob = 2 * b + rblk
                    nc.sync.dma_start(
                        out=out[ob * 128:(ob + 1) * 128,
                                ft * 512:(ft + 1) * 512],
                        in_=ot[:])
                chains.append(emit_a)
                chains.append(emit_b)
        return chains

    def emit_divide(b, qt, avs):
        """Divide by the softmax denominator (row 64 of av) and stage into
        the A2A send buffer. PE-free, and the reciprocal is computed on a
        [128, 4] partition-spread layout (a [1, 512] DVE reciprocal costs
        ~3.3us on one partition; spread it costs ~0.1us). Both heads' PSUM
        evictions run first so the next q-tile's attn@v never stalls on
        the divide chain. All DMAs on the sync HWDGE queue so the
        collective sitting on the gpsimd queue can never block them."""
        oraws = []
        for h in range(HPC):
            # evict the accumulator to SBUF immediately: releases the PSUM
            # slot so the next q-tile's attn@v never waits on this divide
            oraw = smallp.tile([65, QT], BF, tag="oraw", name=f"oraw{b}{h}{qt}")
            nc.vector.tensor_copy(oraw[:], avs[h][:])
            oraws.append(oraw)
        for h in range(HPC):
            oraw = oraws[h]
            # spread the denominator over 4 partitions (4-descriptor DMAs;
            # a [128, x] spread costs 128 descriptors ~ 6us completion)
            den_d = dramp.tile([1, QT], BF, tag="dend", bufs=4,
                               name=f"dend{b}{h}{qt}")
            nc.sync.dma_start(out=den_d[:], in_=oraw[64:65, :])
            den4 = smallp.tile([4, 128], BF, tag="den4",
                               name=f"den4{b}{h}{qt}")
            nc.sync.dma_start(out=den4[:],
                              in_=den_d.rearrange("o (p c) -> (o p) c", p=4))
            rcp4 = smallp.tile([4, 128], BF, tag="rcp4",
                               name=f"rcp4{b}{h}{qt}")
            with nc.allow_low_precision(reason="bf16 softmax denominator"):
                nc.vector.reciprocal(rcp4[:], den4[:])
            rcp_d = dramp.tile([1, QT], BF, tag="rcpd", bufs=4,
                               name=f"rcpd{b}{h}{qt}")
            nc.sync.dma_start(out=rcp_d.rearrange("o (p c) -> (o p) c", p=4),
                              in_=rcp4[:])
            bcs = smallp.tile([64, QT], BF, tag="bcs", name=f"bcs{b}{h}{qt}")
            bcast = bass.AP(tensor=rcp_d.tensor, offset=rcp_d.offset,
                            ap=[[0, 64]] + list(rcp_d.ap[1:]))
            nc.sync.dma_start(out=bcs[:], in_=bcast)
            odiv = smallp.tile([64, QT], BF, tag="odiv", name=f"odiv{b}{h}{qt}")
            nc.vector.tensor_mul(odiv[:], oraw[0:64, :], bcs[:])
            # q-tile qt covers s in [512qt, 512qt+512): half hf = qt // 2,
            # destination cores j = 4*(qt%2) .. +4, 128 columns each.
            # For the final divide the exp stream is over, so h1's sends can
            # ride the idle ACT HWDGE queue and issue in parallel with h0's.
            last = (b == B - 1 and qt == N_QT - 1)
            send_eng = nc.scalar if (h == 1 and last) else nc.sync
            hf = qt // 2
            for jj in range(4):
                j = 4 * (qt % 2) + jj
                send_eng.dma_start(
                    out=send_d[(b, hf)][j, h * 64:(h + 1) * 64, :],
                    in_=odiv[:, jj * 128:(jj + 1) * 128])

    def emit_attention_batch(b, dribble):
        """All 4 q-tiles of a batch as one rolling pipeline over 64+LAG
        (qt, kc) units: scores+exp lead, attn@v trails by LAG units, the
        divide chain fires as each q-tile's accumulation completes.  One
        dribble chain (qkv projection / output projection) is popped every
        other unit to keep the in-order PE stream dense."""
        scale = 1.0 / math.sqrt(HD)
        NU = N_QT * N_KC
        LAG = 5
        pts = {}
        avs = {}
        for u in range(NU + LAG):
            if u < NU:
                qt, kc = divmod(u, N_KC)
                if kc == 0:
                    avs[qt] = [ps_av.tile([65, QT], F32, tag="av",
                                          name=f"av{b}{h}{qt}")
                               for h in range(HPC)]
                sps = ps_sps.tile([128, 2 * QT], F32, tag="sps",
                                  name=f"s{b}{qt}_{kc}")
                for h in range(HPC):
                    hof = h * 64
                    nc.tensor.matmul(
                        sps[:, h * QT:(h + 1) * QT],
                        kT_sb[b][hof:hof + 64, kc * KC:(kc + 1) * KC],
                        qT_sb[b][hof:hof + 64, qt * QT:(qt + 1) * QT],
                        start=True, stop=True)
                pt = pp.tile([128, 2 * QT], BF, tag="p", name=f"p{b}{qt}_{kc}")
                nc.scalar.activation(pt[:], sps[:], EXPF, scale=scale)
                pts[u] = pt
            if u >= LAG:
                j = u - LAG
                qt2, kc2 = divmod(j, N_KC)
                for h in range(HPC):
                    nc.tensor.matmul(avs[qt2][h][:], v_sb[(b, h, kc2)],
                                     pts[j][:, h * QT:(h + 1) * QT],
                                     start=(kc2 == 0), stop=(kc2 == N_KC - 1))
                del pts[j]
                if kc2 == N_KC - 1:
                    emit_divide(b, qt2, avs.pop(qt2))
                    if qt2 == 1:
                        emit_a2a(b, 0)
            # pop every dribble entry whose target unit has arrived
            while dribble and dribble[0][0] <= u:
                dribble.pop(0)[1]()

    def emit_a2a(b, hf):
        nc.gpsimd.collective_compute(
            "AllToAll", mybir.AluOpType.bypass,
            replica_groups=[list(range(N_CORES))],
            ins=[send_d[(b, hf)].opt()], outs=[recv_d[(b, hf)].opt()])

    # ---------------- emission ----------------
    # r-tile 0 of batch 0 runs alone; batch-0 attention starts right after
    # (its first 4 units only need r-tile 0's q/k/v), with r-tiles 1-3 and
    # then batch 1's qkv dribbled into the unit loop at 2 chains/unit so
    # each r-tile completes just before the units that consume it.
    holders = {0: xt0}

    def prefetch(rt):
        def fn():
            if rt not in holders:
                holders[rt] = emit_xt_load(rt)
        return fn

    def get_xt(rt):
        def g():
            if rt not in holders:
                holders[rt] = emit_xt_load(rt)
            return holders[rt]
        return g

    for chain in qkv_chains(0, get_xt(0)):
        chain()
    # warm the collective path (cold-start ~8us); emitted here so the wait on
    # the gpsimd queue never delays the critical first x/weight loads
    cwu_s = dramp.tile([N_CORES, 8], F32, tag="cwus", name="cwu_s")
    cwu_r = dramp.tile([N_CORES, 8], F32, tag="cwur", name="cwu_r")
    nc.sync.dma_start(out=cwu_s.rearrange("c r -> (c r)")[None, :],
                      in_=ones_f32[0:1, 0:64])
    nc.gpsimd.collective_compute(
        "AllToAll", mybir.AluOpType.bypass,
        replica_groups=[list(range(N_CORES))],
        ins=[cwu_s.opt()], outs=[cwu_r.opt()])

    # r-tile 1's x load starts before the w2 load grabs the queue: its first
    # consumers are the dribbled chains at units 0-1
    prefetch(1)()
    # w2 load early: 2 MB bf16, overlaps the batch-0 attention stretch
    w2_sb[0] = consts.tile([128, NEC, E], BF, tag="w2", name="w2_all")
    nc.gpsimd.dma_start(out=w2_sb[0][:],
                        in_=w2T.rearrange("(c p) f -> p c f", p=128))
    # batch-0 attention with b0 r-tiles 1-3 then batch-1 qkv dribbled in
    dribble = []
    for rt in (2, 3):
        dribble.append((4 * (rt - 1) - 3, prefetch(rt)))
    for rt in (1, 2, 3):
        for i, c in enumerate(qkv_chains(rt, get_xt(rt))):
            dribble.append((4 * (rt - 1) + i // 2, c))
    for k, rt in enumerate((4, 5, 6, 7)):
        base = 12 + 14 * k
        dribble.append((base - 3, prefetch(rt)))
        for i, c in enumerate(qkv_chains(rt, get_xt(rt))):
            dribble.append((base + i * 14 // 8, c))
    dribble.sort(key=lambda e: e[0])
    emit_attention_batch(0, dribble)
    for _, chain in dribble:
        chain()
    del dribble[:]
    emit_a2a(0, 1)                     # second half, fires at batch-0 end

    # both batch-0 halves complete early in batch-1 attention;
    # batch-1 half 0's A2A fires mid-batch, its projection runs at the tail
    dribble = [(24 + 2 * i, c)
               for i, c in enumerate(proj_chains(0, 0) + proj_chains(0, 1))]
    dribble += [(58 + 2 * i, c) for i, c in enumerate(proj_chains(1, 0))]
    emit_attention_batch(1, dribble)
    for _, chain in dribble:
        chain()
    emit_a2a(1, 1)
    for chain in proj_chains(1, 1):
        chain()
    ctx.close()


def _host_prep(x, w1, w2):
    import ml_dtypes
    bf16 = ml_dtypes.bfloat16
    x = np.asarray(x, dtype=np.float32)
    w1 = np.asarray(w1, dtype=np.float32)
    w2 = np.asarray(w2, dtype=np.float32)

    xT = np.ascontiguousarray(x.reshape(R, E).T.astype(bf16))   # [E, R]
    w2T = np.ascontiguousarray(w2.T.astype(bf16))               # [E, E]

    theta = 1.0 / (BASE ** (np.arange(0, HD, 2, dtype=np.float32) / HD))
    enc = np.arange(S, dtype=np.float32)[:, None] * theta[None, :]
    enc = np.repeat(enc, 2, axis=-1)                      # [s, 64]
    cos1 = np.cos(enc).T.astype(np.float32)               # [64, S]
    sin1 = np.sin(enc).T.astype(np.float32)
    cosT = np.ascontiguousarray(np.concatenate([cos1, cos1], axis=0).astype(bf16))
    sinT = np.ascontiguousarray(np.concatenate([sin1, sin1], axis=0).astype(bf16))

    m64 = np.zeros((HD, HD), dtype=np.float32)
    for i in range(HD // 2):
        m64[2 * i, 2 * i + 1] = -1.0
        m64[2 * i + 1, 2 * i] = 1.0
    m128 = np.zeros((128, 128), dtype=np.float32)
    m128[:64, :64] = m64
    m128[64:, 64:] = m64
    p2T = np.ascontiguousarray(m128.T.astype(bf16))

    in_maps = []
    for c in range(N_CORES):
        hA, hB = HPC * c, HPC * c + 1
        def rows(base):
            return np.concatenate(
                [w1[base + hA * HD: base + (hA + 1) * HD, :],
                 w1[base + hB * HD: base + (hB + 1) * HD, :]], axis=0)
        in_maps.append({
            "xT": xT,
            "wqT": np.ascontiguousarray(rows(0).T.astype(bf16)),
            "wkT": np.ascontiguousarray(rows(E).T.astype(bf16)),
            "wvT": np.ascontiguousarray(rows(2 * E).T.astype(bf16)),
            "w2T": w2T,
            "cosT": cosT,
            "sinT": sinT,
            "p2T": p2T,
        })
    return in_maps


def kernel(x, w1, w2, _trace=False):
    if "nc" not in _COMPILED:
        _COMPILED["nc"] = _build_nc()
    nc = _COMPILED["nc"]
    in_maps = _host_prep(x, w1, w2)
    res = run_bass_kernel_spmd(nc, in_maps, core_ids=list(range(N_CORES)),
                               trace=_trace)
    _COMPILED["last_result"] = res
    # core c returns [512, E] as four 128-row blocks:
    # [b0 s=128c.., b0 s=1024+128c.., b1 s=128c.., b1 s=1024+128c..]
    full = np.empty((B, S, E), dtype=np.float32)
    for c in range(N_CORES):
        blk = res.results[c]["out"]
        full[0, 128 * c:128 * (c + 1)] = blk[0:128]
        full[0, 1024 + 128 * c:1024 + 128 * (c + 1)] = blk[128:256]
        full[1, 128 * c:128 * (c + 1)] = blk[256:384]
        full[1, 1024 + 128 * c:1024 + 128 * (c + 1)] = blk[384:512]
    return full


# revision 22
# speedup vs baseline: 1.0621x; 1.0157x over previous
"""Trainium2 Bass kernel for nn_Attention_12000138625343.

Full multi-head attention layer (B=2, S=2048, E=1024, H=16, hd=64, interleaved
RoPE on q/k, non-causal softmax) run tensor-parallel over 8 NeuronCores:

  - heads sharded 2-per-core (w1 columns / qkv projection sharded),
  - x replicated, passed pre-transposed [E, B*S] so the contraction dim lands
    on SBUF partitions,
  - all matmul operands in bfloat16 (PSUM accumulation stays fp32): the PE
    runs out of the fp32 power-throttle regime, LDWEIGHTS and every DMA
    halve, and DVE element-wise ops hit their 2x/4x 16-bit modes,
  - scores computed transposed [k, q]; the two heads' K=64 score matmuls are
    packed into disjoint PE row-groups (concurrent), one exp instruction
    covers both heads' [128, 1024] PSUM block,
  - the Scalar (ACT) engine runs *only* the exp stream - every PSUM eviction
    lives on Pool (gpsimd) or DVE so softmax throughput is never stolen,
  - the v projection runs in its fast transposed orientation (N=512) and is
    flipped back to [k, hd] by PE transposes against an identity,
  - attn@v accumulates rolling per k-chunk with a ones-column appended to v
    producing the softmax denominator; the divide runs entirely off the
    TensorEngine (DVE reciprocal + DRAM-bounce broadcast DMA + DVE multiply),
  - batch-1 qkv projection / batch-0 output projection matmul chains are
    dribbled into the attention k-chunk loop so the in-order PE stream never
    starves the exp pipeline for long,
  - four bf16 AllToAlls (one per batch-half, each gated by that half's last
    softmax divide) convert the head sharding of the attention output o^T
    into row sharding; all but the final 0.25 MB collective hide under
    remaining compute, and each w2-projection block runs as soon as its
    half has arrived,
  - each core owns 2 x 128 rows of each batch; host reassembles.
"""

import math

import numpy as np

import concourse.bass as bass
import concourse.mybir as mybir
import concourse.tile as tile
from concourse import bacc
from concourse.bass_utils import run_bass_kernel_spmd
from concourse.masks import make_identity

B, S, E, H = 2, 2048, 1024, 16
HD = E // H  # 64
BASE = 10000.0
N_CORES = 8
HPC = H // N_CORES       # heads per core = 2
R = B * S                # 4096 flattened rows
RT = 512                 # rows per r-tile
NEC = E // 128           # 8 e-chunks of 128
QT = 512                 # q columns per q-tile
N_QT = S // QT           # 4 q-tiles per batch
KC = 128                 # k rows per k-chunk
N_KC = S // KC           # 16 k-chunks per batch
RPB = S // N_CORES       # rows per core per batch = 256

F32 = mybir.dt.float32
BF = mybir.dt.bfloat16
EXPF = mybir.ActivationFunctionType.Exp

_COMPILED = {}


def _build_nc():
    nc = bacc.Bacc("TRN2", target_bir_lowering=False, debug=False,
                   num_devices=N_CORES)

    xT = nc.dram_tensor("xT", [E, R], BF, kind="ExternalInput").ap()
    wqT = nc.dram_tensor("wqT", [E, 128], BF, kind="ExternalInput").ap()
    wkT = nc.dram_tensor("wkT", [E, 128], BF, kind="ExternalInput").ap()
    wvT = nc.dram_tensor("wvT", [E, 128], BF, kind="ExternalInput").ap()
    w2T = nc.dram_tensor("w2T", [E, E], BF, kind="ExternalInput").ap()
    cosT = nc.dram_tensor("cosT", [128, S], BF, kind="ExternalInput").ap()
    sinT = nc.dram_tensor("sinT", [128, S], BF, kind="ExternalInput").ap()
    p2T = nc.dram_tensor("p2T", [128, 128], BF, kind="ExternalInput").ap()
    out = nc.dram_tensor("out", [2 * RPB, E], F32, kind="ExternalOutput").ap()

    with tile.TileContext(nc) as tc:
        _emit(tc, nc, xT, wqT, wkT, wvT, w2T, cosT, sinT, p2T, out)
    nc.compile()
    return nc


def _emit(tc, nc, xT, wqT, wkT, wvT, w2T, cosT, sinT, p2T, out):
    import contextlib
    ctx = contextlib.ExitStack()
    consts = ctx.enter_context(tc.tile_pool(name="consts", bufs=1))
    xtp = ctx.enter_context(tc.tile_pool(name="xtp", bufs=2))
    qkp = ctx.enter_context(tc.tile_pool(name="qkp", bufs=1))
    rawp = ctx.enter_context(tc.tile_pool(name="rawp", bufs=2))
    tmpp = ctx.enter_context(tc.tile_pool(name="tmpp", bufs=2))
    vp = ctx.enter_context(tc.tile_pool(name="vp", bufs=1))
    pp = ctx.enter_context(tc.tile_pool(name="pp", bufs=7))
    smallp = ctx.enter_context(tc.tile_pool(name="smallp", bufs=2))
    dramp = ctx.enter_context(tc.tile_pool(name="dramp", bufs=1, space="DRAM"))
    # PSUM budget (8 banks): qkv-shared 2 + sps 2 x 2 + av 2 = 8
    ps_qkv = ctx.enter_context(tc.tile_pool(name="ps_qkv", bufs=2, space="PSUM"))
    ps_sps = ctx.enter_context(tc.tile_pool(name="ps_sps", bufs=2, space="PSUM"))
    ps_av = ctx.enter_context(tc.tile_pool(name="ps_av", bufs=2, space="PSUM"))

    # ---- batched constant loads (single DMA each, all bf16); q weights +
    # the first half of the first x tile go first so the first matmul can
    # start a few us in ----
    wq_all = consts.tile([128, NEC, 128], BF, tag="wq", name="wq_all")
    nc.sync.dma_start(out=wq_all[:], in_=wqT.rearrange("(c p) f -> p c f", p=128))
    xt0 = xtp.tile([128, NEC, RT], BF, tag="xt", bufs=3, name="xt_0")
    xr = xT.rearrange("(c p) r -> p c r", p=128)
    nc.gpsimd.dma_start(out=xt0[:, 0:4, :], in_=xr[:, 0:4, 0:RT])
    nc.gpsimd.dma_start(out=xt0[:, 4:NEC, :], in_=xr[:, 4:NEC, 0:RT])
    wk_all = consts.tile([128, NEC, 128], BF, tag="wk", name="wk_all")
    nc.gpsimd.dma_start(out=wk_all[:], in_=wkT.rearrange("(c p) f -> p c f", p=128))
    wv_all = consts.tile([128, NEC, 128], BF, tag="wv", name="wv_all")
    nc.gpsimd.dma_start(out=wv_all[:], in_=wvT.rearrange("(c p) f -> p c f", p=128))
    p2_sb = consts.tile([128, 128], BF, tag="p2", name="p2_sb")
    nc.gpsimd.dma_start(out=p2_sb[:], in_=p2T[:, :])

    cos_sb = consts.tile([128, S], BF, tag="cos", name="cos_sb")
    nc.sync.dma_start(out=cos_sb[:], in_=cosT[:, :])
    sin_sb = consts.tile([128, S], BF, tag="sin", name="sin_sb")
    nc.sync.dma_start(out=sin_sb[:], in_=sinT[:, :])
    ones_f32 = consts.tile([1, 64], F32, tag="ones32", name="ones_f32")
    nc.vector.memset(ones_f32[:], 1.0)
    id_sb = consts.tile([128, 128], BF, tag="idm", name="id_sb")
    make_identity(nc, id_sb[:])

    # A2A buffers, one pair per (batch, half): [8 chunks, 128 e-rows, 128 rows]
    # half 0 carries s in [128j, 128j+128) (ready after q-tile 1),
    # half 1 carries s in [1024+128j, ...) (ready after q-tile 3).
    send_d = {(b, hf): dramp.tile([N_CORES, 128, 128], BF, name=f"send{b}{hf}")
              for b in range(B) for hf in range(2)}
    recv_d = {(b, hf): dramp.tile([N_CORES, 128, 128], BF, name=f"recv{b}{hf}")
              for b in range(B) for hf in range(2)}

    qT_sb, kT_sb, v_sb = {}, {}, {}
    w2_sb = {}

    def emit_xt_load(rt):
        if rt == 0:
            return xt0
        t = xtp.tile([128, NEC, RT], BF, tag="xt", bufs=3, name=f"xt_{rt}")
        nc.gpsimd.dma_start(
            out=t[:],
            in_=xr[:, :, rt * RT:(rt + 1) * RT])
        return t

    def qkv_chains(rt, get_xt):
        """Return a list of closures, each emitting one matmul chain (+ its
        epilogue) for r-tile rt. Callers dribble these between attention
        steps to keep the in-order PE stream dense but never monolithic.
        get_xt is called at chain-emission time so the x load DMA can be
        prefetched by an earlier dribble entry."""
        b, st = rt // N_QT, (rt % N_QT) * RT

        if b not in qT_sb:
            qT_sb[b] = qkp.tile([128, S], BF, tag=f"qT{b}", name=f"qT{b}")
            kT_sb[b] = qkp.tile([128, S], BF, tag=f"kT{b}", name=f"kT{b}")

        def qk_chain(kind, w_all, dst):
            state = {}
            def emit_a():
                xt = get_xt()
                acc = ps_qkv.tile([128, RT], F32, tag="qkv",
                                  name=f"{kind}acc{rt}")
                for ec in range(4):
                    nc.tensor.matmul(acc[:], w_all[:, ec, :], xt[:, ec, :],
                                     start=(ec == 0), stop=False)
                state["acc"] = acc
            def emit_b():
                xt = get_xt()
                acc = state.pop("acc")
                for ec in range(4, NEC):
                    nc.tensor.matmul(acc[:], w_all[:, ec, :], xt[:, ec, :],
                                     start=False, stop=(ec == NEC - 1))
                raw = rawp.tile([128, RT], BF, tag="raw",
                                name=f"{kind}raw{rt}")
                nc.vector.tensor_copy(raw[:], acc[:])
                rot = ps_qkv.tile([128, RT], F32, tag="qkv",
                                  name=f"{kind}rot{rt}")
                nc.tensor.matmul(rot[:], p2_sb[:], raw[:], start=True, stop=True)
                t1 = tmpp.tile([128, RT], BF, tag="ropet", name=f"{kind}t1_{rt}")
                nc.vector.tensor_mul(t1[:], raw[:], cos_sb[:, st:st + RT])
                t2 = tmpp.tile([128, RT], BF, tag="ropet", name=f"{kind}t2_{rt}")
                nc.vector.tensor_mul(t2[:], rot[:], sin_sb[:, st:st + RT])
                nc.vector.tensor_add(dst[:, st:st + RT], t1[:], t2[:])
            return [emit_a, emit_b]

        vstate = {}

        def v_head_chain(half):
            # v^T = wv.T @ x computed at full rate (N=512), half the e-chunks
            # per pop; the PE transpose in v_tail_chain flips it back to the
            # [k, hd] layout attn@v needs.
            def emit():
                xt = get_xt()
                if half == 0:
                    vacc = ps_qkv.tile([128, RT], F32, tag="qkv",
                                       name=f"vTacc{rt}")
                    vstate["ps"] = vacc
                vacc = vstate["ps"]
                for ec in range(4 * half, 4 * half + 4):
                    nc.tensor.matmul(vacc[:], wv_all[:, ec, :], xt[:, ec, :],
                                     start=(ec == 0), stop=(ec == NEC - 1))
                if half == 1:
                    vts = rawp.tile([128, RT], BF, tag="raw",
                                    name=f"vts{rt}")
                    nc.vector.tensor_copy(vts[:], vstate.pop("ps")[:])
                    vstate["sb"] = vts
            return emit

        def v_tail_chain(pair):
            def emit():
                vts = vstate["sb"]
                for sub in (2 * pair, 2 * pair + 1):
                    vtr = ps_qkv.tile([128, 128], BF, tag="qkv",
                                      name=f"vtr{rt}_{sub}")
                    nc.tensor.transpose(
                        vtr[:], vts[:, sub * 128:(sub + 1) * 128], id_sb[:])
                    kc = (rt % N_QT) * 4 + sub
                    # both heads' [k, 64] blocks + their ones columns live in
                    # one [128, 130] tile: a single strided-AP copy and a
                    # single strided memset replace 4 per-head DVE ops
                    vt = vp.tile([128, 130], BF, tag=f"v{b}{kc}",
                                 name=f"v{b}{kc}")
                    vt_h = vt[:, 0:130].rearrange("p (h c) -> p h c", c=65)
                    nc.vector.tensor_copy(
                        vt_h[:, :, 0:64],
                        vtr[:, 0:128].rearrange("p (h c) -> p h c", c=64))
                    nc.vector.memset(vt_h[:, :, 64:65], 1.0)
                    for h in range(HPC):
                        v_sb[(b, h, kc)] = vt[:, 65 * h:65 * h + 65]
            return emit

        return qk_chain("q", wq_all, qT_sb[b]) + \
               qk_chain("k", wk_all, kT_sb[b]) + \
               [v_head_chain(0), v_head_chain(1),
                v_tail_chain(0), v_tail_chain(1)]

    def proj_chains(b, hf):
        """Output projection for my 128 rows of (batch b, half hf).
        The recv load is emitted lazily by the first chain so that building
        the chain list never precedes the collective's emission."""
        state0 = {}
        def get_recv():
            if "t" not in state0:
                t = xtp.tile([128, NEC, 128], BF, tag="recv", bufs=2,
                             name=f"recv{b}{hf}")
                nc.gpsimd.dma_start(
                    out=t[:], in_=recv_d[(b, hf)].rearrange("c p r -> p c r"))
                state0["t"] = t
            return state0["t"]
        chains = []
        for rblk in [hf]:
            for ft in range(2):
                state = {}
                def emit_a(rblk=rblk, ft=ft, state=state):
                    recv_sb = get_recv()
                    # qkv psum tag: free during attention (projection is done)
                    ops = ps_qkv.tile([128, 512], F32, tag="qkv",
                                      name=f"ops{b}_{rblk}_{ft}")
                    for ec in range(4):
                        nc.tensor.matmul(
                            ops[:],
                            recv_sb[:, ec, :],
                            w2_sb[0][:, ec, ft * 512:(ft + 1) * 512],
                            start=(ec == 0), stop=False)
                    state["ops"] = ops
                def emit_b(rblk=rblk, ft=ft, state=state):
                    recv_sb = get_recv()
                    ops = state.pop("ops")
                    for ec in range(4, NEC):
                        nc.tensor.matmul(
                            ops[:],
                            recv_sb[:, ec, :],
                            w2_sb[0][:, ec, ft * 512:(ft + 1) * 512],
                            start=False, stop=(ec == NEC - 1))
                    ot = tmpp.tile([128, 512], F32, tag="ropet",
                                   name=f"ot{b}_{rblk}_{ft}")
                    nc.vector.tensor_copy(ot[:], ops[:])
                    # out rows: [b0h0, b0h1, b1h0, b1h1] blocks of 128
                    ob = 2 * b + rblk
                    nc.sync.dma_start(
                        out=out[ob * 128:(ob + 1) * 128,
                                ft * 512:(ft + 1) * 512],
                        in_=ot[:])
                chains.append(emit_a)
                chains.append(emit_b)
        return chains

    def emit_divide(b, qt, avs):
        """Divide by the softmax denominator (row 64 of av) and stage into
        the A2A send buffer. PE-free, and the reciprocal is computed on a
        [128, 4] partition-spread layout (a [1, 512] DVE reciprocal costs
        ~3.3us on one partition; spread it costs ~0.1us). Both heads' PSUM
        evictions run first so the next q-tile's attn@v never stalls on
        the divide chain. All DMAs on the sync HWDGE queue so the
        collective sitting on the gpsimd queue can never block them."""
        oraws = []
        for h in range(HPC):
            # evict the accumulator to SBUF immediately: releases the PSUM
            # slot so the next q-tile's attn@v never waits on this divide
            oraw = smallp.tile([65, QT], BF, tag="oraw", name=f"oraw{b}{h}{qt}")
            nc.vector.tensor_copy(oraw[:], avs[h][:])
            oraws.append(oraw)
        for h in range(HPC):
            oraw = oraws[h]
            # spread the denominator over 4 partitions (4-descriptor DMAs;
            # a [128, x] spread costs 128 descriptors ~ 6us completion)
            den_d = dramp.tile([1, QT], BF, tag="dend", bufs=4,
                               name=f"dend{b}{h}{qt}")
            nc.sync.dma_start(out=den_d[:], in_=oraw[64:65, :])
            den4 = smallp.tile([4, 128], BF, tag="den4",
                               name=f"den4{b}{h}{qt}")
            nc.sync.dma_start(out=den4[:],
                              in_=den_d.rearrange("o (p c) -> (o p) c", p=4))
            rcp4 = smallp.tile([4, 128], BF, tag="rcp4",
                               name=f"rcp4{b}{h}{qt}")
            with nc.allow_low_precision(reason="bf16 softmax denominator"):
                nc.vector.reciprocal(rcp4[:], den4[:])
            rcp_d = dramp.tile([1, QT], BF, tag="rcpd", bufs=4,
                               name=f"rcpd{b}{h}{qt}")
            nc.sync.dma_start(out=rcp_d.rearrange("o (p c) -> (o p) c", p=4),
                              in_=rcp4[:])
            bcs = smallp.tile([64, QT], BF, tag="bcs", name=f"bcs{b}{h}{qt}")
            bcast = bass.AP(tensor=rcp_d.tensor, offset=rcp_d.offset,
                            ap=[[0, 64]] + list(rcp_d.ap[1:]))
            nc.sync.dma_start(out=bcs[:], in_=bcast)
            odiv = smallp.tile([64, QT], BF, tag="odiv", name=f"odiv{b}{h}{qt}")
            nc.vector.tensor_mul(odiv[:], oraw[0:64, :], bcs[:])
            # q-tile qt covers s in [512qt, 512qt+512): half hf = qt // 2,
            # destination cores j = 4*(qt%2) .. +4, 128 columns each.
            # For the final divide the exp stream is over, so h1's sends can
            # ride the idle ACT HWDGE queue and issue in parallel with h0's.
            last = (b == B - 1 and qt == N_QT - 1)
            send_eng = nc.scalar if (h == 1 and last) else nc.sync
            hf = qt // 2
            for jj in range(4):
                j = 4 * (qt % 2) + jj
                send_eng.dma_start(
                    out=send_d[(b, hf)][j, h * 64:(h + 1) * 64, :],
                    in_=odiv[:, jj * 128:(jj + 1) * 128])

    def emit_attention_batch(b, dribble):
        """All 4 q-tiles of a batch as one rolling pipeline over 64+LAG
        (qt, kc) units: scores+exp lead, attn@v trails by LAG units, the
        divide chain fires as each q-tile's accumulation completes.  One
        dribble chain (qkv projection / output projection) is popped every
        other unit to keep the in-order PE stream dense."""
        scale = 1.0 / math.sqrt(HD)
        NU = N_QT * N_KC
        LAG = 5
        pts = {}
        avs = {}
        for u in range(NU + LAG):
            if u < NU:
                qt, kc = divmod(u, N_KC)
                if kc == 0:
                    avs[qt] = [ps_av.tile([65, QT], F32, tag="av",
                                          name=f"av{b}{h}{qt}")
                               for h in range(HPC)]
                sps = ps_sps.tile([128, 2 * QT], F32, tag="sps",
                                  name=f"s{b}{qt}_{kc}")
                for h in range(HPC):
                    hof = h * 64
                    nc.tensor.matmul(
                        sps[:, h * QT:(h + 1) * QT],
                        kT_sb[b][hof:hof + 64, kc * KC:(kc + 1) * KC],
                        qT_sb[b][hof:hof + 64, qt * QT:(qt + 1) * QT],
                        start=True, stop=True)
                pt = pp.tile([128, 2 * QT], BF, tag="p", name=f"p{b}{qt}_{kc}")
                nc.scalar.activation(pt[:], sps[:], EXPF, scale=scale)
                pts[u] = pt
            if u >= LAG:
                j = u - LAG
                qt2, kc2 = divmod(j, N_KC)
                for h in range(HPC):
                    nc.tensor.matmul(avs[qt2][h][:], v_sb[(b, h, kc2)],
                                     pts[j][:, h * QT:(h + 1) * QT],
                                     start=(kc2 == 0), stop=(kc2 == N_KC - 1))
                del pts[j]
                if kc2 == N_KC - 1:
                    emit_divide(b, qt2, avs.pop(qt2))
                    if qt2 == 1:
                        emit_a2a(b, 0)
            # pop every dribble entry whose target unit has arrived
            while dribble and dribble[0][0] <= u:
                dribble.pop(0)[1]()

    def emit_a2a(b, hf):
        nc.gpsimd.collective_compute(
            "AllToAll", mybir.AluOpType.bypass,
            replica_groups=[list(range(N_CORES))],
            ins=[send_d[(b, hf)].opt()], outs=[recv_d[(b, hf)].opt()])

    # ---------------- emission ----------------
    # r-tile 0 of batch 0 runs alone; batch-0 attention starts right after
    # (its first 4 units only need r-tile 0's q/k/v), with r-tiles 1-3 and
    # then batch 1's qkv dribbled into the unit loop at 2 chains/unit so
    # each r-tile completes just before the units that consume it.
    holders = {0: xt0}

    def prefetch(rt):
        def fn():
            if rt not in holders:
                holders[rt] = emit_xt_load(rt)
        return fn

    def get_xt(rt):
        def g():
            if rt not in holders:
                holders[rt] = emit_xt_load(rt)
            return holders[rt]
        return g

    for chain in qkv_chains(0, get_xt(0)):
        chain()
    # warm the collective path (cold-start ~8us); emitted here so the wait on
    # the gpsimd queue never delays the critical first x/weight loads
    cwu_s = dramp.tile([N_CORES, 8], F32, tag="cwus", name="cwu_s")
    cwu_r = dramp.tile([N_CORES, 8], F32, tag="cwur", name="cwu_r")
    nc.sync.dma_start(out=cwu_s.rearrange("c r -> (c r)")[None, :],
                      in_=ones_f32[0:1, 0:64])
    nc.gpsimd.collective_compute(
        "AllToAll", mybir.AluOpType.bypass,
        replica_groups=[list(range(N_CORES))],
        ins=[cwu_s.opt()], outs=[cwu_r.opt()])

    # r-tile 1's x load starts before the w2 load grabs the queue: its first
    # consumers are the dribbled chains at units 0-1
    prefetch(1)()
    # w2 load early: 2 MB bf16, overlaps the batch-0 attention stretch
    w2_sb[0] = consts.tile([128, NEC, E], BF, tag="w2", name="w2_all")
    nc.gpsimd.dma_start(out=w2_sb[0][:],
                        in_=w2T.rearrange("(c p) f -> p c f", p=128))
    # batch-0 attention with b0 r-tiles 1-3 then batch-1 qkv dribbled in
    dribble = []
    for rt in (2, 3):
        dribble.append((4 * (rt - 1) - 3, prefetch(rt)))
    for rt in (1, 2, 3):
        for i, c in enumerate(qkv_chains(rt, get_xt(rt))):
            dribble.append((4 * (rt - 1) + i // 2, c))
    for k, rt in enumerate((4, 5, 6, 7)):
        base = 12 + 14 * k
        dribble.append((base - 3, prefetch(rt)))
        for i, c in enumerate(qkv_chains(rt, get_xt(rt))):
            dribble.append((base + i * 14 // 8, c))
    dribble.sort(key=lambda e: e[0])
    emit_attention_batch(0, dribble)
    for _, chain in dribble:
        chain()
    del dribble[:]
    emit_a2a(0, 1)                     # second half, fires at batch-0 end

    # both batch-0 halves complete early in batch-1 attention;
    # batch-1 half 0's A2A fires mid-batch, its projection runs at the tail.
    # proj(0,1)'s A2A only fires at b0's end - give it ~45 units of slack so
    # a slow/skewed collective never stalls the PE queue.
    dribble = [(24 + 2 * i, c) for i, c in enumerate(proj_chains(0, 0))]
    dribble += [(44 + 2 * i, c) for i, c in enumerate(proj_chains(0, 1))]
    dribble += [(60 + 2 * i, c) for i, c in enumerate(proj_chains(1, 0))]
    emit_attention_batch(1, dribble)
    for _, chain in dribble:
        chain()
    emit_a2a(1, 1)
    for chain in proj_chains(1, 1):
        chain()
    ctx.close()


def _host_prep(x, w1, w2):
    import ml_dtypes
    bf16 = ml_dtypes.bfloat16
    x = np.asarray(x, dtype=np.float32)
    w1 = np.asarray(w1, dtype=np.float32)
    w2 = np.asarray(w2, dtype=np.float32)

    xT = np.ascontiguousarray(x.reshape(R, E).T.astype(bf16))   # [E, R]
    w2T = np.ascontiguousarray(w2.T.astype(bf16))               # [E, E]

    theta = 1.0 / (BASE ** (np.arange(0, HD, 2, dtype=np.float32) / HD))
    enc = np.arange(S, dtype=np.float32)[:, None] * theta[None, :]
    enc = np.repeat(enc, 2, axis=-1)                      # [s, 64]
    cos1 = np.cos(enc).T.astype(np.float32)               # [64, S]
    sin1 = np.sin(enc).T.astype(np.float32)
    cosT = np.ascontiguousarray(np.concatenate([cos1, cos1], axis=0).astype(bf16))
    sinT = np.ascontiguousarray(np.concatenate([sin1, sin1], axis=0).astype(bf16))

    m64 = np.zeros((HD, HD), dtype=np.float32)
    for i in range(HD // 2):
        m64[2 * i, 2 * i + 1] = -1.0
        m64[2 * i + 1, 2 * i] = 1.0
    m128 = np.zeros((128, 128), dtype=np.float32)
    m128[:64, :64] = m64
    m128[64:, 64:] = m64
    p2T = np.ascontiguousarray(m128.T.astype(bf16))

    in_maps = []
    for c in range(N_CORES):
        hA, hB = HPC * c, HPC * c + 1
        def rows(base):
            return np.concatenate(
                [w1[base + hA * HD: base + (hA + 1) * HD, :],
                 w1[base + hB * HD: base + (hB + 1) * HD, :]], axis=0)
        in_maps.append({
            "xT": xT,
            "wqT": np.ascontiguousarray(rows(0).T.astype(bf16)),
            "wkT": np.ascontiguousarray(rows(E).T.astype(bf16)),
            "wvT": np.ascontiguousarray(rows(2 * E).T.astype(bf16)),
            "w2T": w2T,
            "cosT": cosT,
            "sinT": sinT,
            "p2T": p2T,
        })
    return in_maps


def kernel(x, w1, w2, _trace=False):
    if "nc" not in _COMPILED:
        _COMPILED["nc"] = _build_nc()
    nc = _COMPILED["nc"]
    in_maps = _host_prep(x, w1, w2)
    res = run_bass_kernel_spmd(nc, in_maps, core_ids=list(range(N_CORES)),
                               trace=_trace)
    _COMPILED["last_result"] = res
    # core c returns [512, E] as four 128-row blocks:
    # [b0 s=128c.., b0 s=1024+128c.., b1 s=128c.., b1 s=1024+128c..]
    full = np.empty((B, S, E), dtype=np.float32)
    for c in range(N_CORES):
        blk = res.results[c]["out"]
        full[0, 128 * c:128 * (c + 1)] = blk[0:128]
        full[0, 1024 + 128 * c:1024 + 128 * (c + 1)] = blk[128:256]
        full[1, 128 * c:128 * (c + 1)] = blk[256:384]
        full[1, 1024 + 128 * c:1024 + 128 * (c + 1)] = blk[384:512]
    return full
